# revision 18
# baseline (speedup 1.0000x reference)
"""Trainium2 Bass kernel for nn_ReaReaConv (GCN-style message passing with
dynamic edge gating).

Math (per batch b):
    deg[n]   = in-degree(n) + 1 (self loop);  dis = rsqrt(deg)
    f_e      = keep*fdo + (1-keep)*(1-fdo), keep = sigmoid(2*flux[src]*flux[tgt])
    out[t]   = dis_t * ( (T-V)[t] @ Wc^T + V[t] @ Wd^T ) + bias
    T[t]     = sum_{e->t} dis_src * x[src_e]  + dis_t * x[t]   (self loop)
    V[t]     = sum_{e->t} dis_src * f_e * x[src_e]             (f_self = 0)

Sharding: each of the 8 cores owns N/8 target nodes (tiles of T=96). Host
sorts edges by (target tile, gather table, target) - indices/layout only;
all FP arithmetic runs on device. x is staged as a bf16 gather table (pure
byte truncation of fp32, a layout transform); edge-aggregation matmuls run
in bf16 with fp32 PSUM accumulation.

Key structure ("window one-hots"): within a tile the edges are target-sorted,
so each 128-edge chunk spans only a few consecutive targets. Every chunk gets
a uniform W=16 one-hot window (host picks the window base; splits a chunk in
the rare case its span exceeds W). Per super-tile, four merged DVE ops build
eq = is_equal(tl_rel, iota_W) and the three scaled copies eq*g, eq*(g*f0),
eq*(g*f1). Per chunk, three matmuls accumulate into PSUM column slices:
T += x_chunk @ (eq*g), V_b += x_chunk[b] @ (eq*g*f_b) (col-tiled M=64).
Self loops never enter the edge stream; a per-tile diagonal matmul adds
dis_t * x[t] to T (and zeroes the accumulator via start=True).

Gathers are batched per super-tile (3 tiles) into 4 dma_gather calls spread
over the 4 SWDGE queues. Per-tile/super-tile shapes are exact (maxed over the
8 cores) and baked into the program; builds are cached per shape signature.
"""

from dataclasses import dataclass

import ml_dtypes
import numpy as np

# -------------------- problem constants --------------------
N_NODES = 50000
N_EDGES = 1600000
BATCH = 2
C = 64
N_CORES = 8
TILE = 96            # target nodes per tile
W = 24               # one-hot window width per chunk
GSUP = 3             # tiles per super-tile (gather batching)
CHUNK = 128          # edges per matmul chunk (PE contraction)
SPLIT = 32768        # gather-table split (int16 signed index limit)

NPC = N_NODES // N_CORES          # nodes per core
NTC = -(-NPC // TILE)             # tiles per core (last one ragged)
NSUP = -(-NTC // GSUP)            # super-tiles per core


def bf16_trunc(a):
    """fp32 -> bf16 by dropping the low 16 bits (pure byte-layout transform)."""
    a = np.ascontiguousarray(np.asarray(a, np.float32))
    hi = (a.view(np.uint32) >> 16).astype(np.uint16)
    return hi.view(ml_dtypes.bfloat16)


@dataclass(frozen=True)
class Cfg:
    # per (super, tile-in-super) chunk counts, maxed over cores
    cta: tuple     # A-table chunks, flattened [NSUP*GSUP]
    ctb: tuple     # B-table chunks
    tn: tuple      # per-tile target count [NTC]
    base: tuple    # per chunk-slot window base, flattened in column order

    @property
    def sct(self):  # chunks per super-tile
        out = []
        for s in range(NSUP):
            sl = slice(s * GSUP, (s + 1) * GSUP)
            out.append(sum(self.cta[sl]) + sum(self.ctb[sl]))
        return tuple(out)

    @property
    def soff(self):  # super-tile column offsets
        o, acc = [], 0
        for c in self.sct:
            o.append(acc)
            acc += c
        return tuple(o)

    @property
    def ctn(self):
        return sum(self.sct)

    @property
    def na(self):
        return SPLIT

    @property
    def nb(self):
        return N_NODES - SPLIT


# -------------------- host prep (indices / layout only) --------------------

def _wrap16(idx_flat):
    """dma_gather index layout: [128, n/16] int16, idx[p, s] = flat[s*16+p],
    replicated across the 8 gpsimd cores (partition blocks of 16)."""
    n = len(idx_flat)
    assert n % 16 == 0
    w = np.asarray(idx_flat, np.int16).reshape(n // 16, 16).T
    return np.tile(w, (8, 1))


def _sorted_edges(edge_index):
    """Original edges only (no self loops), sorted by target."""
    src0 = np.asarray(edge_index[0]).astype(np.int64)
    tgt0 = np.asarray(edge_index[1]).astype(np.int64)
    perm = np.argsort(tgt0, kind="stable")
    return src0, tgt0, src0[perm], tgt0[perm], perm


def build_layout(edge_index):
    """Lockstep chunk schedule shared by all 8 cores.

    Per (tile, table-group), all cores pack their target-sorted edges into a
    common sequence of chunk slots.  Slot k's window base is the minimum
    "next target" over cores that still have edges (the laggard position);
    each core then takes up to CHUNK edges with target < base + W.  W absorbs
    both the per-chunk target span and the cross-core position spread, so
    slot counts stay within one of each core's own optimum.

    Returns (slots, assign):
      slots[tt] = (basesA, basesB) - window base per chunk slot
      assign[core][tt] = (selA, selB) - per slot, the selected positions into
          the core's tile-local edge order (indices into ga/gb subsets)
    """
    _, _, src_s, tgt_s, _ = _sorted_edges(edge_index)
    bounds = [tt * TILE for tt in range(NTC)] + [NPC]
    seg = []  # [core][tt] -> (ga, gb, tg) tile-local data
    for c in range(N_CORES):
        base = c * NPC
        starts = np.searchsorted(tgt_s, [base + b for b in bounds])
        tiles = []
        for tt in range(NTC):
            s, e = starts[tt], starts[tt + 1]
            seg_src = src_s[s:e]
            seg_tgt = tgt_s[s:e] - (base + tt * TILE)
            isa = seg_src < SPLIT
            ga = np.nonzero(isa)[0]
            gb = np.nonzero(~isa)[0]
            tiles.append((s, ga, seg_tgt[ga], gb, seg_tgt[gb]))
        seg.append(tiles)

    slots = []
    assign = [[None] * NTC for _ in range(N_CORES)]
    for tt in range(NTC):
        per_group = []
        sels = [[None, None] for _ in range(N_CORES)]
        for gi, (gsel_i, tg_i) in enumerate(((1, 2), (3, 4))):
            ptr = [0] * N_CORES
            tgs = [seg[c][tt][tg_i] for c in range(N_CORES)]
            bases = []
            core_sel = [[] for _ in range(N_CORES)]
            while True:
                nxt = [tgs[c][ptr[c]] for c in range(N_CORES)
                       if ptr[c] < len(tgs[c])]
                if not nxt:
                    break
                base = min(TILE - W, int(min(nxt)))
                bases.append(base)
                for c in range(N_CORES):
                    i0 = ptr[c]
                    j = min(i0 + CHUNK, len(tgs[c]))
                    hi = np.searchsorted(tgs[c][i0:j], base + W, side="left")
                    j = i0 + int(hi)
                    core_sel[c].append((i0, j))
                    ptr[c] = j
            per_group.append(tuple(bases))
            for c in range(N_CORES):
                sels[c][gi] = core_sel[c]
        slots.append((per_group[0], per_group[1]))
        for c in range(N_CORES):
            assign[c][tt] = (sels[c][0], sels[c][1])
    return seg, slots, assign


def prep(x, edge_index, f_disc_orig, fluxes):
    """Returns (cfg, shared dict, per-core dicts). Indices / layout /
    dtype-layout only - no floating-point arithmetic."""
    n = N_NODES
    src0, tgt0, src_s, tgt_s, perm = _sorted_edges(edge_index)
    x = np.asarray(x, np.float32)
    fdo_in = np.asarray(f_disc_orig, np.float32)
    fluxes = np.asarray(fluxes, np.float32)

    deg = (np.bincount(tgt0, minlength=n) + 1).astype(np.float32)

    per_edge_all = np.stack([
        fdo_in, fluxes[0][src0], fluxes[1][src0],
        fluxes[0][tgt0], fluxes[1][tgt0], deg[src0],
    ])  # [6, E]: fdo, fs0, fs1, ft0, ft1, degs
    per_edge = per_edge_all[:, perm]

    seg, slots, assign = build_layout(edge_index)

    # column layout: per super-tile: [A(t0) A(t1) A(t2) B(t0) B(t1) B(t2)]
    cta = [len(slots[tt][0]) for tt in range(NTC)]
    ctb = [len(slots[tt][1]) for tt in range(NTC)]
    tn = tuple(min(TILE, NPC - tt * TILE) for tt in range(NTC))
    col_of = {}
    bases = []
    acc = 0
    for s in range(NSUP):
        tset = range(s * GSUP, min((s + 1) * GSUP, NTC))
        for grp, gi in (("a", 0), ("b", 1)):
            for tt in tset:
                col_of[(tt, grp)] = acc
                bases.extend(slots[tt][gi])
                acc += len(slots[tt][gi])
    ctn = acc
    cfg = Cfg(cta=tuple(cta), ctb=tuple(ctb), tn=tn, base=tuple(bases))

    xcat = np.concatenate([x[0], x[1]], axis=1)  # [n, 128]
    iota_w = np.tile(np.arange(W, dtype=np.float32), (128, 1))
    id96 = np.zeros((128, TILE), np.float32)
    id96[np.arange(TILE), np.arange(TILE)] = 1.0
    shared = {
        "xpa": bf16_trunc(xcat[:SPLIT]),
        "xpb": bf16_trunc(xcat[SPLIT:]),
        "iotaw": bf16_trunc(iota_w),
        "id96": bf16_trunc(id96),
        "zeros96": np.zeros((128, TILE), ml_dtypes.bfloat16),
        "zeros128": np.zeros((128, 128), ml_dtypes.bfloat16),
    }

    names = ["fdo", "fs0", "fs1", "ft0", "ft1"]
    cores = []
    for core in range(N_CORES):
        base_node = core * NPC
        tl_all = np.full((128, ctn), -1.0, np.float32)
        pe_all = np.zeros((6, 128, ctn), np.float32)
        pe_all[5] = 1.0
        idx16 = np.zeros((128, ctn * 8), np.int16)
        degown = np.ones((128, NTC), np.float32)
        degrow = np.ones((1, NTC * TILE), np.float32)
        xq = np.zeros((128, NTC * 128), np.float32)
        for tt in range(NTC):
            s, ga, tga, gb, tgb = seg[core][tt]
            selA, selB = assign[core][tt]
            for grp, gsel, tg, chunks, idoff in (
                    ("a", ga, tga, selA, 0), ("b", gb, tgb, selB, SPLIT)):
                c0 = col_of[(tt, grp)]
                sbases = slots[tt][0 if grp == "a" else 1]
                for k, (i0, i1) in enumerate(chunks):
                    nn = i1 - i0
                    if nn == 0:
                        continue
                    col = c0 + k
                    b = sbases[k]
                    sel = gsel[i0:i1]
                    tl_all[:nn, col] = tg[i0:i1] - b
                    for j in range(6):
                        pe_all[j][:nn, col] = per_edge[j][s + sel]
                    pe_all[5][nn:, col] = 1.0
                    ids = np.zeros(CHUNK, np.int64)
                    ids[:nn] = src_s[s + sel] - idoff
                    idx16[:, col * 8:(col + 1) * 8] = _wrap16(ids)

            tnn = tn[tt]
            t0g = base_node + tt * TILE
            degown[:tnn, tt] = deg[t0g:t0g + tnn]
            degrow[0, tt * TILE:tt * TILE + tnn] = deg[t0g:t0g + tnn]
            xq[:tnn, tt * 128:tt * 128 + 128] = xcat[t0g:t0g + tnn]
        d = {"tl": bf16_trunc(tl_all), "idx16": idx16,
             "degown": degown, "degrow": degrow,
             "degs": np.ascontiguousarray(pe_all[5]),
             "xq": bf16_trunc(xq)}
        for j, nm in enumerate(names):
            d[nm] = bf16_trunc(pe_all[j])
        cores.append(d)
    return cfg, shared, cores


# -------------------- device program --------------------

def build_nc(cfg: Cfg):
    import concourse.tile as tile
    from concourse import bacc, mybir

    dt = mybir.dt
    act = mybir.ActivationFunctionType
    alu = mybir.AluOpType

    ctn = cfg.ctn
    T = TILE

    nc = bacc.Bacc("TRN2", target_bir_lowering=False, debug=False,
                   num_swdge_queues=4)

    xpa = nc.dram_tensor("xpa", [cfg.na, 2 * C], dt.bfloat16, kind="ExternalInput")
    xpb = nc.dram_tensor("xpb", [cfg.nb, 2 * C], dt.bfloat16, kind="ExternalInput")
    tl_d = nc.dram_tensor("tl", [128, ctn], dt.bfloat16, kind="ExternalInput")
    fdo_d = nc.dram_tensor("fdo", [128, ctn], dt.bfloat16, kind="ExternalInput")
    fs0_d = nc.dram_tensor("fs0", [128, ctn], dt.bfloat16, kind="ExternalInput")
    fs1_d = nc.dram_tensor("fs1", [128, ctn], dt.bfloat16, kind="ExternalInput")
    ft0_d = nc.dram_tensor("ft0", [128, ctn], dt.bfloat16, kind="ExternalInput")
    ft1_d = nc.dram_tensor("ft1", [128, ctn], dt.bfloat16, kind="ExternalInput")
    degs_d = nc.dram_tensor("degs", [128, ctn], dt.float32, kind="ExternalInput")
    idx16_d = nc.dram_tensor("idx16", [128, ctn * 8], dt.int16, kind="ExternalInput")
    degown_d = nc.dram_tensor("degown", [128, NTC], dt.float32, kind="ExternalInput")
    degrow_d = nc.dram_tensor("degrow", [1, NTC * T], dt.float32, kind="ExternalInput")
    iotaw_d = nc.dram_tensor("iotaw", [128, W], dt.bfloat16, kind="ExternalInput")
    id96_d = nc.dram_tensor("id96", [128, T], dt.bfloat16, kind="ExternalInput")
    z96_d = nc.dram_tensor("zeros96", [128, T], dt.bfloat16, kind="ExternalInput")
    z128_d = nc.dram_tensor("zeros128", [128, 128], dt.bfloat16, kind="ExternalInput")
    xq_d = nc.dram_tensor("xq", [128, NTC * 128], dt.bfloat16, kind="ExternalInput")
    wct_d = nc.dram_tensor("wct2", [128, C], dt.bfloat16, kind="ExternalInput")
    wdt_d = nc.dram_tensor("wdt2", [128, C], dt.bfloat16, kind="ExternalInput")
    bias_d = nc.dram_tensor("bias2", [1, 2 * C], dt.bfloat16, kind="ExternalInput")
    outB = nc.dram_tensor("outB", [NPC, 2 * C], dt.float32, kind="ExternalOutput")

    with tile.TileContext(nc) as tc:
        with (
            tc.tile_pool(name="const", bufs=1) as constp,
            tc.tile_pool(name="res", bufs=1) as resp,
        ):
            iotaw_sb = constp.tile([128, W], dt.bfloat16)
            nc.sync.dma_start(iotaw_sb[:], iotaw_d[:, :])
            id96_sb = constp.tile([128, T], dt.bfloat16)
            nc.sync.dma_start(id96_sb[:], id96_d[:, :])
            z96_sb = constp.tile([128, T], dt.bfloat16)
            nc.sync.dma_start(z96_sb[:], z96_d[:, :])
            z128_sb = constp.tile([128, 128], dt.bfloat16)
            nc.sync.dma_start(z128_sb[:], z128_d[:, :])
            wct_sb = constp.tile([128, C], dt.bfloat16)
            nc.sync.dma_start(wct_sb[:], wct_d[:, :])
            wdt_sb = constp.tile([128, C], dt.bfloat16)
            nc.sync.dma_start(wdt_sb[:], wdt_d[:, :])
            bias_sb = constp.tile([1, 2 * C], dt.bfloat16)
            nc.sync.dma_start(bias_sb[:], bias_d[:, :])

            disown_sb = constp.tile([128, NTC], dt.float32)
            nc.sync.dma_start(disown_sb[:], degown_d[:, :])
            nc.vector.reciprocal(disown_sb[:], disown_sb[:])
            nc.scalar.activation(disown_sb[:], disown_sb[:], act.Sqrt)
            sqdeg_sb = constp.tile([1, NTC * T], dt.bfloat16)
            sqf_sb = constp.tile([1, NTC * T], dt.float32)
            nc.sync.dma_start(sqf_sb[:], degrow_d[:, :])
            nc.scalar.activation(sqdeg_sb[:], sqf_sb[:], act.Sqrt)

            dxall = resp.tile([128, NTC * T], dt.bfloat16)
            # warm-up gather: loads the Q7 gather ucode + inits rings while
            # the input DMAs stream, off the critical path
            wuidx = resp.tile([128, 8], dt.int16)
            nc.gpsimd.memset(wuidx[:], 0)
            for q in range(4):
                wug = resp.tile([128, 2 * C], dt.bfloat16, tag=f"wug{q}",
                                name=f"wug{q}")
                nc.gpsimd.dma_gather(
                    wug[:].rearrange("p (c r) -> p c r", r=2 * C),
                    xpa[:, :], wuidx[:], CHUNK, CHUNK, 2 * C,
                    single_packet=False, queue_num=q)
            # idx per super-tile as separate tiles, so each super's gathers
            # depend only on their own slice's load
            _soff = cfg.soff
            _sct = cfg.sct
            idx_tiles = []
            for _s in range(NSUP):
                it = resp.tile([128, _sct[_s] * 8], dt.int16,
                               tag=f"idx{_s}", name=f"idx{_s}")
                nc.sync.dma_start(
                    it[:], idx16_d[:, _soff[_s] * 8:(_soff[_s] + _sct[_s]) * 8])
                idx_tiles.append(it)
            tl_sb = resp.tile([128, ctn], dt.bfloat16)
            nc.sync.dma_start(tl_sb[:], tl_d[:, :])
            xq_sb = resp.tile([128, NTC * 128], dt.bfloat16)
            nc.sync.dma_start(xq_sb[:], xq_d[:, :])
            g_sb = resp.tile([128, ctn], dt.bfloat16)
            gf0_sb = resp.tile([128, ctn], dt.bfloat16)
            gf1_sb = resp.tile([128, ctn], dt.bfloat16)

            # ---- prepass: g = rsqrt(deg_src), gf_b = g * f_b ----
            with tc.tile_pool(name="pp", bufs=1) as ppp:
                grec = ppp.tile([128, ctn], dt.float32)
                nc.sync.dma_start(grec[:], degs_d[:, :])
                nc.vector.reciprocal(grec[:], grec[:])
                nc.scalar.activation(g_sb[:], grec[:], act.Sqrt)

                fdo_sb = ppp.tile([128, ctn], dt.bfloat16)
                nc.sync.dma_start(fdo_sb[:], fdo_d[:, :])
                c1 = ppp.tile([128, ctn], dt.bfloat16)
                nc.vector.tensor_scalar(
                    c1[:], fdo_sb[:], 2.0, -1.0, alu.mult, alu.add)
                c0 = ppp.tile([128, ctn], dt.bfloat16)
                nc.vector.tensor_scalar(
                    c0[:], fdo_sb[:], -1.0, 1.0, alu.mult, alu.add)
                for b, (fsd, ftd, gfd) in enumerate(
                        ((fs0_d, ft0_d, gf0_sb), (fs1_d, ft1_d, gf1_sb))):
                    fs_sb = ppp.tile([128, ctn], dt.bfloat16, tag="fs")
                    nc.sync.dma_start(fs_sb[:], fsd[:, :])
                    ft_sb = ppp.tile([128, ctn], dt.bfloat16, tag="ft")
                    nc.sync.dma_start(ft_sb[:], ftd[:, :])
                    nc.vector.tensor_mul(fs_sb[:], fs_sb[:], ft_sb[:])
                    nc.scalar.activation(
                        ft_sb[:], fs_sb[:], act.Sigmoid, scale=2.0)
                    nc.vector.tensor_mul(ft_sb[:], ft_sb[:], c1[:])
                    nc.vector.tensor_add(ft_sb[:], ft_sb[:], c0[:])
                    nc.vector.tensor_mul(gfd[:], ft_sb[:], g_sb[:])
                for tt in range(NTC):
                    nc.vector.tensor_scalar(
                        dxall[:T, tt * T:(tt + 1) * T], id96_sb[:T, :],
                        disown_sb[:T, tt:tt + 1], None, alu.mult)

            # ---- main loop over super-tiles / tiles ----
            with (
                tc.tile_pool(name="xg", bufs=2) as xgp,
                tc.tile_pool(name="oe", bufs=2) as oep,
                tc.tile_pool(name="uv", bufs=4) as uvp,
                tc.tile_pool(name="outp", bufs=4) as outsp,
                tc.tile_pool(name="ps_tv", bufs=3, space="PSUM") as pstv,
                tc.tile_pool(name="ps_o", bufs=2, space="PSUM") as pso,
            ):
                qrr = 0
                soff = cfg.soff
                for s in range(NSUP):
                    tset = list(range(s * GSUP, min((s + 1) * GSUP, NTC)))
                    sct = cfg.sct[s]
                    scta = sum(cfg.cta[tt] for tt in tset)
                    sc0 = soff[s]
                    ib = sc0 * 8

                    xg = xgp.tile([128, sct * 2 * C], dt.bfloat16, tag="xg")
                    xg3 = xg[:].rearrange("p (c r) -> p c r", r=2 * C)
                    calls = [(0, scta // 2, xpa), (scta // 2, scta, xpa),
                             (scta, scta + (sct - scta) // 2, xpb),
                             (scta + (sct - scta) // 2, sct, xpb)]
                    for (c0_, c1_, tbl) in calls:
                        nch = c1_ - c0_
                        if nch == 0:
                            continue
                        nc.gpsimd.dma_gather(
                            xg3[:, c0_:c1_],
                            tbl[:, :],
                            idx_tiles[s][:, c0_ * 8: c1_ * 8],
                            nch * CHUNK, nch * CHUNK, 2 * C,
                            single_packet=False, queue_num=qrr % 4,
                        )
                        qrr += 1

                    # merged window one-hots for the whole super-tile
                    eqt = oep.tile([128, sct * W], dt.bfloat16, tag="eq")
                    e3 = eqt[:].rearrange("p (c w) -> p c w", w=W)
                    tl_cols = tl_sb[:, sc0:sc0 + sct].unsqueeze(2)
                    nc.vector.tensor_tensor(
                        e3,
                        tl_cols.to_broadcast([128, sct, W]),
                        iotaw_sb[:].unsqueeze(1).to_broadcast([128, sct, W]),
                        alu.is_equal,
                    )
                    ohg = oep.tile([128, sct * W], dt.bfloat16, tag="ohg")
                    ohf0 = oep.tile([128, sct * W], dt.bfloat16, tag="ohf0")
                    ohf1 = oep.tile([128, sct * W], dt.bfloat16, tag="ohf1")
                    for dst, src_cols in ((ohg, g_sb), (ohf0, gf0_sb),
                                          (ohf1, gf1_sb)):
                        d3 = dst[:].rearrange("p (c w) -> p c w", w=W)
                        sc = src_cols[:, sc0:sc0 + sct].unsqueeze(2)
                        nc.vector.tensor_tensor(
                            d3, e3, sc.to_broadcast([128, sct, W]), alu.mult)

                    for tt in tset:
                        tnn = cfg.tn[tt]
                        # chunk columns of this tile within the super layout
                        ca, cb = cfg.cta[tt], cfg.ctb[tt]
                        a0 = sum(cfg.cta[t2] for t2 in tset if t2 < tt)
                        b0 = scta + sum(cfg.ctb[t2] for t2 in tset if t2 < tt)
                        cols = ([a0 + k for k in range(ca)] +
                                [b0 + k for k in range(cb)])

                        t_ps = pstv.tile([128, T], dt.float32, tag="t_ps")
                        v_ps = pstv.tile([128, T], dt.float32, tag="v_ps")

                        # diagonal self-loop matmul zeroes + seeds T:
                        # dx[p, t] = delta(p==t) * dis_t (prebuilt)
                        nc.tensor.matmul(
                            out=t_ps[:],
                            lhsT=xq_sb[:T, tt * 128:(tt + 1) * 128],
                            rhs=dxall[:T, tt * T:(tt + 1) * T],
                            start=True, stop=False)
                        # V accumulator: zero it with a zeros matmul
                        nc.tensor.matmul(
                            out=v_ps[:], lhsT=z128_sb[:], rhs=z96_sb[:],
                            start=True, stop=False)

                        for ci, col in enumerate(cols):
                            base = cfg.base[sc0 + col]
                            lsl = xg[:, col * 2 * C:(col + 1) * 2 * C]
                            wsl = slice(col * W, (col + 1) * W)
                            psl = slice(base, base + W)
                            nc.tensor.matmul(
                                out=t_ps[:, psl], lhsT=lsl, rhs=ohg[:, wsl],
                                start=False, stop=(ci == len(cols) - 1))
                            nc.tensor.matmul(
                                out=v_ps[0:64, psl], lhsT=lsl[:, 0:C],
                                rhs=ohf0[:, wsl], start=False, stop=False,
                                tile_position=(0, 0))
                            nc.tensor.matmul(
                                out=v_ps[64:128, psl], lhsT=lsl[:, C:2 * C],
                                rhs=ohf1[:, wsl], start=False,
                                stop=(ci == len(cols) - 1),
                                tile_position=(0, 64))

                        # epilogue
                        vm = uvp.tile([128, T], dt.bfloat16, tag="vm")
                        nc.scalar.activation(vm[:], v_ps[:], act.Copy)
                        um = uvp.tile([128, T], dt.bfloat16, tag="um")
                        nc.vector.tensor_tensor(um[:], t_ps[:], vm[:],
                                                alu.subtract)

                        op_ps = pso.tile([T, 2 * C], dt.float32, tag="op")
                        sq_row = sqdeg_sb[0:1, tt * T: tt * T + T]
                        for bi in range(2):
                            rows = slice(C * bi, C * bi + C)
                            csl = slice(C * bi, C * bi + C)
                            nc.tensor.matmul(
                                out=op_ps[:, csl], lhsT=um[rows, :],
                                rhs=wct_sb[rows, :], start=True, stop=False)
                            nc.tensor.matmul(
                                out=op_ps[:, csl], lhsT=vm[rows, :],
                                rhs=wdt_sb[rows, :], start=False, stop=False)
                            nc.tensor.matmul(
                                out=op_ps[:, csl], lhsT=sq_row,
                                rhs=bias_sb[0:1, csl], start=False, stop=True)

                        o_sb = outsp.tile([128, 2 * C], dt.float32, tag="os")
                        nc.vector.tensor_scalar(
                            o_sb[:T, :], op_ps[:, :], disown_sb[:T, tt:tt + 1],
                            None, alu.mult)
                        nc.sync.dma_start(
                            outB[tt * T:tt * T + tnn, :], o_sb[:tnn, :])

    nc.compile()
    return nc


def _shared_weights(W_conc, W_disc, bias):
    wct2 = np.zeros((128, C), np.float32)
    wdt2 = np.zeros((128, C), np.float32)
    wct2[:64] = np.asarray(W_conc, np.float32).T
    wct2[64:] = wct2[:64]
    wdt2[:64] = np.asarray(W_disc, np.float32).T
    wdt2[64:] = wdt2[:64]
    bias2 = np.tile(np.asarray(bias, np.float32)[None, :], (1, 2))
    return bf16_trunc(wct2), bf16_trunc(wdt2), bf16_trunc(bias2)


_NC_CACHE = {}


def _run(inputs, trace=False):
    from concourse.bass_utils import run_bass_kernel_spmd

    x = np.asarray(inputs["x"], np.float32)
    cfg, shared, cores = prep(x, inputs["edge_index"],
                              inputs["f_disc_orig"], inputs["fluxes"])
    wct2, wdt2, bias2 = _shared_weights(
        inputs["W_conc"], inputs["W_disc"], inputs["bias"])
    in_maps = []
    for core in range(N_CORES):
        m = dict(shared)
        m.update(cores[core])
        m["wct2"] = wct2
        m["wdt2"] = wdt2
        m["bias2"] = bias2
        in_maps.append(m)

    if cfg not in _NC_CACHE:
        _NC_CACHE[cfg] = build_nc(cfg)
    nc = _NC_CACHE[cfg]

    res = run_bass_kernel_spmd(nc, in_maps, list(range(N_CORES)),
                               trace=trace)
    out = np.zeros((BATCH, N_NODES, C), np.float32)
    for core in range(N_CORES):
        ob = res.results[core]["outB"]
        out[0, core * NPC:(core + 1) * NPC] = ob[:, :C]
        out[1, core * NPC:(core + 1) * NPC] = ob[:, C:]
    return out, res


def kernel(x, edge_index, f_disc_orig, fluxes, W_conc, W_disc, bias):
    out, _ = _run(dict(x=x, edge_index=edge_index, f_disc_orig=f_disc_orig,
                       fluxes=fluxes, W_conc=W_conc, W_disc=W_disc, bias=bias))
    return out


def profile_run(inputs):
    out, res = _run(inputs, trace=True)
    return res.exec_time_ns


# revision 19
# speedup vs baseline: 1.1448x; 1.1448x over previous
"""Trainium2 Bass kernel for nn_ReaReaConv (GCN-style message passing with
dynamic edge gating).

Math (per batch b):
    deg[n]   = in-degree(n) + 1 (self loop);  dis = rsqrt(deg)
    f_e      = keep*fdo + (1-keep)*(1-fdo), keep = sigmoid(2*flux[src]*flux[tgt])
    out[t]   = dis_t * ( (T-V)[t] @ Wc^T + V[t] @ Wd^T ) + bias
    T[t]     = sum_{e->t} dis_src * x[src_e]  + dis_t * x[t]   (self loop)
    V[t]     = sum_{e->t} dis_src * f_e * x[src_e]             (f_self = 0)

Sharding: each of the 8 cores owns N/8 target nodes (tiles of T=96). Host
sorts edges by (target tile, gather table, target) - indices/layout only;
all FP arithmetic runs on device. x is staged as a bf16 gather table (pure
byte truncation of fp32, a layout transform); edge-aggregation matmuls run
in bf16 with fp32 PSUM accumulation.

Key structure ("window one-hots"): within a tile the edges are target-sorted,
so each 128-edge chunk spans only a few consecutive targets. Every chunk gets
a uniform W=16 one-hot window (host picks the window base; splits a chunk in
the rare case its span exceeds W). Per super-tile, four merged DVE ops build
eq = is_equal(tl_rel, iota_W) and the three scaled copies eq*g, eq*(g*f0),
eq*(g*f1). Per chunk, three matmuls accumulate into PSUM column slices:
T += x_chunk @ (eq*g), V_b += x_chunk[b] @ (eq*g*f_b) (col-tiled M=64).
Self loops never enter the edge stream; a per-tile diagonal matmul adds
dis_t * x[t] to T (and zeroes the accumulator via start=True).

Gathers are batched per super-tile (3 tiles) into 4 dma_gather calls spread
over the 4 SWDGE queues. Per-tile/super-tile shapes are exact (maxed over the
8 cores) and baked into the program; builds are cached per shape signature.
"""

from dataclasses import dataclass

import ml_dtypes
import numpy as np

# -------------------- problem constants --------------------
N_NODES = 50000
N_EDGES = 1600000
BATCH = 2
C = 64
N_CORES = 8
TILE = 96            # target nodes per tile
W = 24               # one-hot window width per chunk
GSUP = 3             # tiles per super-tile (gather batching)
CHUNK = 128          # edges per matmul chunk (PE contraction)
SPLIT = 32768        # gather-table split (int16 signed index limit)

NPC = N_NODES // N_CORES          # nodes per core
NTC = -(-NPC // TILE)             # tiles per core (last one ragged)
NSUP = -(-NTC // GSUP)            # super-tiles per core


def bf16_trunc(a):
    """fp32 -> bf16 by dropping the low 16 bits (pure byte-layout transform)."""
    a = np.ascontiguousarray(np.asarray(a, np.float32))
    hi = (a.view(np.uint32) >> 16).astype(np.uint16)
    return hi.view(ml_dtypes.bfloat16)


@dataclass(frozen=True)
class Cfg:
    # per (super, tile-in-super) chunk counts, maxed over cores
    cta: tuple     # A-table chunks, flattened [NSUP*GSUP]
    ctb: tuple     # B-table chunks
    tn: tuple      # per-tile target count [NTC]
    base: tuple    # per chunk-slot window base, flattened in column order

    @property
    def sct(self):  # chunks per super-tile
        out = []
        for s in range(NSUP):
            sl = slice(s * GSUP, (s + 1) * GSUP)
            out.append(sum(self.cta[sl]) + sum(self.ctb[sl]))
        return tuple(out)

    @property
    def soff(self):  # super-tile column offsets
        o, acc = [], 0
        for c in self.sct:
            o.append(acc)
            acc += c
        return tuple(o)

    @property
    def ctn(self):
        return sum(self.sct)

    @property
    def na(self):
        return SPLIT

    @property
    def nb(self):
        return N_NODES - SPLIT


# -------------------- host prep (indices / layout only) --------------------

def _wrap16(idx_flat):
    """dma_gather index layout: [128, n/16] int16, idx[p, s] = flat[s*16+p],
    replicated across the 8 gpsimd cores (partition blocks of 16)."""
    n = len(idx_flat)
    assert n % 16 == 0
    w = np.asarray(idx_flat, np.int16).reshape(n // 16, 16).T
    return np.tile(w, (8, 1))


def _sorted_edges(edge_index):
    """Original edges only (no self loops), sorted by target."""
    src0 = np.asarray(edge_index[0]).astype(np.int64)
    tgt0 = np.asarray(edge_index[1]).astype(np.int64)
    perm = np.argsort(tgt0, kind="stable")
    return src0, tgt0, src0[perm], tgt0[perm], perm


def build_layout(edge_index):
    """Lockstep chunk schedule shared by all 8 cores.

    Per (tile, table-group), all cores pack their target-sorted edges into a
    common sequence of chunk slots.  Slot k's window base is the minimum
    "next target" over cores that still have edges (the laggard position);
    each core then takes up to CHUNK edges with target < base + W.  W absorbs
    both the per-chunk target span and the cross-core position spread, so
    slot counts stay within one of each core's own optimum.

    Returns (slots, assign):
      slots[tt] = (basesA, basesB) - window base per chunk slot
      assign[core][tt] = (selA, selB) - per slot, the selected positions into
          the core's tile-local edge order (indices into ga/gb subsets)
    """
    _, _, src_s, tgt_s, _ = _sorted_edges(edge_index)
    bounds = [tt * TILE for tt in range(NTC)] + [NPC]
    seg = []  # [core][tt] -> (ga, gb, tg) tile-local data
    for c in range(N_CORES):
        base = c * NPC
        starts = np.searchsorted(tgt_s, [base + b for b in bounds])
        tiles = []
        for tt in range(NTC):
            s, e = starts[tt], starts[tt + 1]
            seg_src = src_s[s:e]
            seg_tgt = tgt_s[s:e] - (base + tt * TILE)
            isa = seg_src < SPLIT
            ga = np.nonzero(isa)[0]
            gb = np.nonzero(~isa)[0]
            tiles.append((s, ga, seg_tgt[ga], gb, seg_tgt[gb]))
        seg.append(tiles)

    slots = []
    assign = [[None] * NTC for _ in range(N_CORES)]
    for tt in range(NTC):
        per_group = []
        sels = [[None, None] for _ in range(N_CORES)]
        for gi, (gsel_i, tg_i) in enumerate(((1, 2), (3, 4))):
            ptr = [0] * N_CORES
            tgs = [seg[c][tt][tg_i] for c in range(N_CORES)]
            bases = []
            core_sel = [[] for _ in range(N_CORES)]
            while True:
                nxt = [tgs[c][ptr[c]] for c in range(N_CORES)
                       if ptr[c] < len(tgs[c])]
                if not nxt:
                    break
                base = min(TILE - W, int(min(nxt)))
                bases.append(base)
                for c in range(N_CORES):
                    i0 = ptr[c]
                    j = min(i0 + CHUNK, len(tgs[c]))
                    hi = np.searchsorted(tgs[c][i0:j], base + W, side="left")
                    j = i0 + int(hi)
                    core_sel[c].append((i0, j))
                    ptr[c] = j
            per_group.append(tuple(bases))
            for c in range(N_CORES):
                sels[c][gi] = core_sel[c]
        slots.append((per_group[0], per_group[1]))
        for c in range(N_CORES):
            assign[c][tt] = (sels[c][0], sels[c][1])
    return seg, slots, assign


def prep(x, edge_index, f_disc_orig, fluxes):
    """Returns (cfg, shared dict, per-core dicts). Indices / layout /
    dtype-layout only - no floating-point arithmetic."""
    n = N_NODES
    src0, tgt0, src_s, tgt_s, perm = _sorted_edges(edge_index)
    x = np.asarray(x, np.float32)
    fdo_in = np.asarray(f_disc_orig, np.float32)
    fluxes = np.asarray(fluxes, np.float32)

    deg = (np.bincount(tgt0, minlength=n) + 1).astype(np.float32)

    per_edge_all = np.stack([
        fdo_in, fluxes[0][src0], fluxes[1][src0],
        fluxes[0][tgt0], fluxes[1][tgt0], deg[src0],
    ])  # [6, E]: fdo, fs0, fs1, ft0, ft1, degs
    per_edge = per_edge_all[:, perm]

    seg, slots, assign = build_layout(edge_index)

    # column layout: per super-tile: [A(t0) A(t1) A(t2) B(t0) B(t1) B(t2)]
    cta = [len(slots[tt][0]) for tt in range(NTC)]
    ctb = [len(slots[tt][1]) for tt in range(NTC)]
    tn = tuple(min(TILE, NPC - tt * TILE) for tt in range(NTC))
    col_of = {}
    bases = []
    acc = 0
    for s in range(NSUP):
        tset = range(s * GSUP, min((s + 1) * GSUP, NTC))
        for grp, gi in (("a", 0), ("b", 1)):
            for tt in tset:
                col_of[(tt, grp)] = acc
                bases.extend(slots[tt][gi])
                acc += len(slots[tt][gi])
    ctn = acc
    cfg = Cfg(cta=tuple(cta), ctb=tuple(ctb), tn=tn, base=tuple(bases))

    xcat = np.concatenate([x[0], x[1]], axis=1)  # [n, 128]
    iota_w = np.tile(np.arange(W, dtype=np.float32), (128, 1))
    id96 = np.zeros((128, TILE), np.float32)
    id96[np.arange(TILE), np.arange(TILE)] = 1.0
    shared = {
        "xpa": bf16_trunc(xcat[:SPLIT]),
        "xpb": bf16_trunc(xcat[SPLIT:]),
        "iotaw": bf16_trunc(iota_w),
        "id96": bf16_trunc(id96),
        "zeros96": np.zeros((128, TILE), ml_dtypes.bfloat16),
        "zeros128": np.zeros((128, 128), ml_dtypes.bfloat16),
    }

    names = ["fdo", "fs0", "fs1", "ft0", "ft1"]
    cores = []
    for core in range(N_CORES):
        base_node = core * NPC
        tl_all = np.full((128, ctn), -1.0, np.float32)
        pe_all = np.zeros((6, 128, ctn), np.float32)
        pe_all[5] = 1.0
        idx16 = np.zeros((128, ctn * 8), np.int16)
        degown = np.ones((128, NTC), np.float32)
        degrow = np.ones((1, NTC * TILE), np.float32)
        xq = np.zeros((128, NTC * 128), np.float32)
        for tt in range(NTC):
            s, ga, tga, gb, tgb = seg[core][tt]
            selA, selB = assign[core][tt]
            for grp, gsel, tg, chunks, idoff in (
                    ("a", ga, tga, selA, 0), ("b", gb, tgb, selB, SPLIT)):
                c0 = col_of[(tt, grp)]
                sbases = slots[tt][0 if grp == "a" else 1]
                for k, (i0, i1) in enumerate(chunks):
                    nn = i1 - i0
                    if nn == 0:
                        continue
                    col = c0 + k
                    b = sbases[k]
                    sel = gsel[i0:i1]
                    tl_all[:nn, col] = tg[i0:i1] - b
                    for j in range(6):
                        pe_all[j][:nn, col] = per_edge[j][s + sel]
                    pe_all[5][nn:, col] = 1.0
                    ids = np.zeros(CHUNK, np.int64)
                    ids[:nn] = src_s[s + sel] - idoff
                    idx16[:, col * 8:(col + 1) * 8] = _wrap16(ids)

            tnn = tn[tt]
            t0g = base_node + tt * TILE
            degown[:tnn, tt] = deg[t0g:t0g + tnn]
            degrow[0, tt * TILE:tt * TILE + tnn] = deg[t0g:t0g + tnn]
            xq[:tnn, tt * 128:tt * 128 + 128] = xcat[t0g:t0g + tnn]
        d = {"tl": bf16_trunc(tl_all), "idx16": idx16,
             "degown": degown, "degrow": degrow,
             "degs": np.ascontiguousarray(pe_all[5]),
             "xq": bf16_trunc(xq)}
        for j, nm in enumerate(names):
            d[nm] = bf16_trunc(pe_all[j])
        cores.append(d)
    return cfg, shared, cores


# -------------------- device program --------------------

def build_nc(cfg: Cfg):
    import concourse.tile as tile
    from concourse import bacc, mybir

    dt = mybir.dt
    act = mybir.ActivationFunctionType
    alu = mybir.AluOpType

    ctn = cfg.ctn
    T = TILE

    nc = bacc.Bacc("TRN2", target_bir_lowering=False, debug=False,
                   num_swdge_queues=4)

    xpa = nc.dram_tensor("xpa", [cfg.na, 2 * C], dt.bfloat16, kind="ExternalInput")
    xpb = nc.dram_tensor("xpb", [cfg.nb, 2 * C], dt.bfloat16, kind="ExternalInput")
    tl_d = nc.dram_tensor("tl", [128, ctn], dt.bfloat16, kind="ExternalInput")
    fdo_d = nc.dram_tensor("fdo", [128, ctn], dt.bfloat16, kind="ExternalInput")
    fs0_d = nc.dram_tensor("fs0", [128, ctn], dt.bfloat16, kind="ExternalInput")
    fs1_d = nc.dram_tensor("fs1", [128, ctn], dt.bfloat16, kind="ExternalInput")
    ft0_d = nc.dram_tensor("ft0", [128, ctn], dt.bfloat16, kind="ExternalInput")
    ft1_d = nc.dram_tensor("ft1", [128, ctn], dt.bfloat16, kind="ExternalInput")
    degs_d = nc.dram_tensor("degs", [128, ctn], dt.float32, kind="ExternalInput")
    idx16_d = nc.dram_tensor("idx16", [128, ctn * 8], dt.int16, kind="ExternalInput")
    degown_d = nc.dram_tensor("degown", [128, NTC], dt.float32, kind="ExternalInput")
    degrow_d = nc.dram_tensor("degrow", [1, NTC * T], dt.float32, kind="ExternalInput")
    iotaw_d = nc.dram_tensor("iotaw", [128, W], dt.bfloat16, kind="ExternalInput")
    id96_d = nc.dram_tensor("id96", [128, T], dt.bfloat16, kind="ExternalInput")
    z96_d = nc.dram_tensor("zeros96", [128, T], dt.bfloat16, kind="ExternalInput")
    z128_d = nc.dram_tensor("zeros128", [128, 128], dt.bfloat16, kind="ExternalInput")
    xq_d = nc.dram_tensor("xq", [128, NTC * 128], dt.bfloat16, kind="ExternalInput")
    wct_d = nc.dram_tensor("wct2", [128, C], dt.bfloat16, kind="ExternalInput")
    wdt_d = nc.dram_tensor("wdt2", [128, C], dt.bfloat16, kind="ExternalInput")
    bias_d = nc.dram_tensor("bias2", [1, 2 * C], dt.bfloat16, kind="ExternalInput")
    outB = nc.dram_tensor("outB", [NPC, 2 * C], dt.float32, kind="ExternalOutput")

    with tile.TileContext(nc) as tc:
        with (
            tc.tile_pool(name="const", bufs=1) as constp,
            tc.tile_pool(name="res", bufs=1) as resp,
        ):
            iotaw_sb = constp.tile([128, W], dt.bfloat16)
            nc.sync.dma_start(iotaw_sb[:], iotaw_d[:, :])
            id96_sb = constp.tile([128, T], dt.bfloat16)
            nc.sync.dma_start(id96_sb[:], id96_d[:, :])
            z96_sb = constp.tile([128, T], dt.bfloat16)
            nc.sync.dma_start(z96_sb[:], z96_d[:, :])
            z128_sb = constp.tile([128, 128], dt.bfloat16)
            nc.sync.dma_start(z128_sb[:], z128_d[:, :])
            wct_sb = constp.tile([128, C], dt.bfloat16)
            nc.sync.dma_start(wct_sb[:], wct_d[:, :])
            wdt_sb = constp.tile([128, C], dt.bfloat16)
            nc.sync.dma_start(wdt_sb[:], wdt_d[:, :])
            bias_sb = constp.tile([1, 2 * C], dt.bfloat16)
            nc.sync.dma_start(bias_sb[:], bias_d[:, :])

            disown_sb = constp.tile([128, NTC], dt.float32)
            nc.sync.dma_start(disown_sb[:], degown_d[:, :])
            nc.vector.reciprocal(disown_sb[:], disown_sb[:])
            nc.scalar.activation(disown_sb[:], disown_sb[:], act.Sqrt)
            sqdeg_sb = constp.tile([1, NTC * T], dt.bfloat16)
            sqf_sb = constp.tile([1, NTC * T], dt.float32)
            nc.sync.dma_start(sqf_sb[:], degrow_d[:, :])
            nc.scalar.activation(sqdeg_sb[:], sqf_sb[:], act.Sqrt)

            dxall = resp.tile([128, NTC * T], dt.bfloat16)
            # warm-up gather: loads the Q7 gather ucode + inits rings while
            # the input DMAs stream, off the critical path
            with tc.high_priority():
                wuidx = resp.tile([128, 8], dt.int16)
                nc.gpsimd.memset(wuidx[:], 0)
                for q in range(4):
                    wug = resp.tile([128, 2 * C], dt.bfloat16, tag=f"wug{q}",
                                    name=f"wug{q}")
                    nc.gpsimd.dma_gather(
                        wug[:].rearrange("p (c r) -> p c r", r=2 * C),
                        xpa[:, :], wuidx[:], CHUNK, CHUNK, 2 * C,
                        single_packet=False, queue_num=q)
                # idx per super-tile as separate tiles, so each super's
                # gathers depend only on their own slice's load; priority 0
                # puts these DMAs ahead of the bulk metadata loads
                _soff = cfg.soff
                _sct = cfg.sct
                idx_tiles = []
                for _s in range(NSUP):
                    it = resp.tile([128, _sct[_s] * 8], dt.int16,
                                   tag=f"idx{_s}", name=f"idx{_s}")
                    nc.sync.dma_start(
                        it[:],
                        idx16_d[:, _soff[_s] * 8:(_soff[_s] + _sct[_s]) * 8])
                    idx_tiles.append(it)
            tl_sb = resp.tile([128, ctn], dt.bfloat16)
            nc.sync.dma_start(tl_sb[:], tl_d[:, :])
            xq_sb = resp.tile([128, NTC * 128], dt.bfloat16)
            nc.sync.dma_start(xq_sb[:], xq_d[:, :])
            g_sb = resp.tile([128, ctn], dt.bfloat16)
            gf0_sb = resp.tile([128, ctn], dt.bfloat16)
            gf1_sb = resp.tile([128, ctn], dt.bfloat16)

            # ---- prepass: g = rsqrt(deg_src), gf_b = g * f_b ----
            with tc.tile_pool(name="pp", bufs=1) as ppp:
                grec = ppp.tile([128, ctn], dt.float32)
                nc.sync.dma_start(grec[:], degs_d[:, :])
                nc.vector.reciprocal(grec[:], grec[:])
                nc.scalar.activation(g_sb[:], grec[:], act.Sqrt)

                fdo_sb = ppp.tile([128, ctn], dt.bfloat16)
                nc.sync.dma_start(fdo_sb[:], fdo_d[:, :])
                c1 = ppp.tile([128, ctn], dt.bfloat16)
                nc.vector.tensor_scalar(
                    c1[:], fdo_sb[:], 2.0, -1.0, alu.mult, alu.add)
                c0 = ppp.tile([128, ctn], dt.bfloat16)
                nc.vector.tensor_scalar(
                    c0[:], fdo_sb[:], -1.0, 1.0, alu.mult, alu.add)
                for b, (fsd, ftd, gfd) in enumerate(
                        ((fs0_d, ft0_d, gf0_sb), (fs1_d, ft1_d, gf1_sb))):
                    fs_sb = ppp.tile([128, ctn], dt.bfloat16, tag="fs")
                    nc.sync.dma_start(fs_sb[:], fsd[:, :])
                    ft_sb = ppp.tile([128, ctn], dt.bfloat16, tag="ft")
                    nc.sync.dma_start(ft_sb[:], ftd[:, :])
                    nc.vector.tensor_mul(fs_sb[:], fs_sb[:], ft_sb[:])
                    nc.scalar.activation(
                        ft_sb[:], fs_sb[:], act.Sigmoid, scale=2.0)
                    nc.vector.tensor_mul(ft_sb[:], ft_sb[:], c1[:])
                    nc.vector.tensor_add(ft_sb[:], ft_sb[:], c0[:])
                    nc.vector.tensor_mul(gfd[:], ft_sb[:], g_sb[:])
                for tt in range(NTC):
                    nc.vector.tensor_scalar(
                        dxall[:T, tt * T:(tt + 1) * T], id96_sb[:T, :],
                        disown_sb[:T, tt:tt + 1], None, alu.mult)

            # ---- main loop over super-tiles / tiles ----
            with (
                tc.tile_pool(name="xg", bufs=3) as xgp,
                tc.tile_pool(name="oe", bufs=2) as oep,
                tc.tile_pool(name="uv", bufs=4) as uvp,
                tc.tile_pool(name="outp", bufs=4) as outsp,
                tc.tile_pool(name="ps_tv", bufs=3, space="PSUM") as pstv,
                tc.tile_pool(name="ps_o", bufs=2, space="PSUM") as pso,
            ):
                qrr = 0
                soff = cfg.soff
                for s in range(NSUP):
                    tset = list(range(s * GSUP, min((s + 1) * GSUP, NTC)))
                    sct = cfg.sct[s]
                    scta = sum(cfg.cta[tt] for tt in tset)
                    sc0 = soff[s]
                    ib = sc0 * 8

                    xg = xgp.tile([128, sct * 2 * C], dt.bfloat16, tag="xg")
                    xg3 = xg[:].rearrange("p (c r) -> p c r", r=2 * C)
                    calls = [(0, scta // 2, xpa), (scta // 2, scta, xpa),
                             (scta, scta + (sct - scta) // 2, xpb),
                             (scta + (sct - scta) // 2, sct, xpb)]
                    for (c0_, c1_, tbl) in calls:
                        nch = c1_ - c0_
                        if nch == 0:
                            continue
                        nc.gpsimd.dma_gather(
                            xg3[:, c0_:c1_],
                            tbl[:, :],
                            idx_tiles[s][:, c0_ * 8: c1_ * 8],
                            nch * CHUNK, nch * CHUNK, 2 * C,
                            single_packet=False, queue_num=qrr % 4,
                        )
                        qrr += 1

                    # merged window one-hots for the whole super-tile
                    eqt = oep.tile([128, sct * W], dt.bfloat16, tag="eq")
                    e3 = eqt[:].rearrange("p (c w) -> p c w", w=W)
                    tl_cols = tl_sb[:, sc0:sc0 + sct].unsqueeze(2)
                    nc.vector.tensor_tensor(
                        e3,
                        tl_cols.to_broadcast([128, sct, W]),
                        iotaw_sb[:].unsqueeze(1).to_broadcast([128, sct, W]),
                        alu.is_equal,
                    )
                    ohg = oep.tile([128, sct * W], dt.bfloat16, tag="ohg")
                    ohf0 = oep.tile([128, sct * W], dt.bfloat16, tag="ohf0")
                    ohf1 = oep.tile([128, sct * W], dt.bfloat16, tag="ohf1")
                    for dst, src_cols in ((ohg, g_sb), (ohf0, gf0_sb),
                                          (ohf1, gf1_sb)):
                        d3 = dst[:].rearrange("p (c w) -> p c w", w=W)
                        sc = src_cols[:, sc0:sc0 + sct].unsqueeze(2)
                        nc.vector.tensor_tensor(
                            d3, e3, sc.to_broadcast([128, sct, W]), alu.mult)

                    for tt in tset:
                        tnn = cfg.tn[tt]
                        # chunk columns of this tile within the super layout
                        ca, cb = cfg.cta[tt], cfg.ctb[tt]
                        a0 = sum(cfg.cta[t2] for t2 in tset if t2 < tt)
                        b0 = scta + sum(cfg.ctb[t2] for t2 in tset if t2 < tt)
                        cols = ([a0 + k for k in range(ca)] +
                                [b0 + k for k in range(cb)])

                        t_ps = pstv.tile([128, T], dt.float32, tag="t_ps")
                        v_ps = pstv.tile([128, T], dt.float32, tag="v_ps")

                        # diagonal self-loop matmul zeroes + seeds T:
                        # dx[p, t] = delta(p==t) * dis_t (prebuilt)
                        nc.tensor.matmul(
                            out=t_ps[:],
                            lhsT=xq_sb[:T, tt * 128:(tt + 1) * 128],
                            rhs=dxall[:T, tt * T:(tt + 1) * T],
                            start=True, stop=False)
                        # V accumulator: zero it with a zeros matmul
                        nc.tensor.matmul(
                            out=v_ps[:], lhsT=z128_sb[:], rhs=z96_sb[:],
                            start=True, stop=False)

                        for ci, col in enumerate(cols):
                            base = cfg.base[sc0 + col]
                            lsl = xg[:, col * 2 * C:(col + 1) * 2 * C]
                            wsl = slice(col * W, (col + 1) * W)
                            psl = slice(base, base + W)
                            nc.tensor.matmul(
                                out=t_ps[:, psl], lhsT=lsl, rhs=ohg[:, wsl],
                                start=False, stop=(ci == len(cols) - 1))
                            nc.tensor.matmul(
                                out=v_ps[0:64, psl], lhsT=lsl[:, 0:C],
                                rhs=ohf0[:, wsl], start=False, stop=False,
                                tile_position=(0, 0))
                            nc.tensor.matmul(
                                out=v_ps[64:128, psl], lhsT=lsl[:, C:2 * C],
                                rhs=ohf1[:, wsl], start=False,
                                stop=(ci == len(cols) - 1),
                                tile_position=(0, 64))

                        # epilogue
                        vm = uvp.tile([128, T], dt.bfloat16, tag="vm")
                        nc.scalar.activation(vm[:], v_ps[:], act.Copy)
                        um = uvp.tile([128, T], dt.bfloat16, tag="um")
                        nc.vector.tensor_tensor(um[:], t_ps[:], vm[:],
                                                alu.subtract)

                        op_ps = pso.tile([T, 2 * C], dt.float32, tag="op")
                        sq_row = sqdeg_sb[0:1, tt * T: tt * T + T]
                        for bi in range(2):
                            rows = slice(C * bi, C * bi + C)
                            csl = slice(C * bi, C * bi + C)
                            nc.tensor.matmul(
                                out=op_ps[:, csl], lhsT=um[rows, :],
                                rhs=wct_sb[rows, :], start=True, stop=False)
                            nc.tensor.matmul(
                                out=op_ps[:, csl], lhsT=vm[rows, :],
                                rhs=wdt_sb[rows, :], start=False, stop=False)
                            nc.tensor.matmul(
                                out=op_ps[:, csl], lhsT=sq_row,
                                rhs=bias_sb[0:1, csl], start=False, stop=True)

                        o_sb = outsp.tile([128, 2 * C], dt.float32, tag="os")
                        nc.vector.tensor_scalar(
                            o_sb[:T, :], op_ps[:, :], disown_sb[:T, tt:tt + 1],
                            None, alu.mult)
                        nc.sync.dma_start(
                            outB[tt * T:tt * T + tnn, :], o_sb[:tnn, :])

    nc.compile()
    return nc


def _shared_weights(W_conc, W_disc, bias):
    wct2 = np.zeros((128, C), np.float32)
    wdt2 = np.zeros((128, C), np.float32)
    wct2[:64] = np.asarray(W_conc, np.float32).T
    wct2[64:] = wct2[:64]
    wdt2[:64] = np.asarray(W_disc, np.float32).T
    wdt2[64:] = wdt2[:64]
    bias2 = np.tile(np.asarray(bias, np.float32)[None, :], (1, 2))
    return bf16_trunc(wct2), bf16_trunc(wdt2), bf16_trunc(bias2)


_NC_CACHE = {}


def _run(inputs, trace=False):
    from concourse.bass_utils import run_bass_kernel_spmd

    x = np.asarray(inputs["x"], np.float32)
    cfg, shared, cores = prep(x, inputs["edge_index"],
                              inputs["f_disc_orig"], inputs["fluxes"])
    wct2, wdt2, bias2 = _shared_weights(
        inputs["W_conc"], inputs["W_disc"], inputs["bias"])
    in_maps = []
    for core in range(N_CORES):
        m = dict(shared)
        m.update(cores[core])
        m["wct2"] = wct2
        m["wdt2"] = wdt2
        m["bias2"] = bias2
        in_maps.append(m)

    if cfg not in _NC_CACHE:
        _NC_CACHE[cfg] = build_nc(cfg)
    nc = _NC_CACHE[cfg]

    res = run_bass_kernel_spmd(nc, in_maps, list(range(N_CORES)),
                               trace=trace)
    out = np.zeros((BATCH, N_NODES, C), np.float32)
    for core in range(N_CORES):
        ob = res.results[core]["outB"]
        out[0, core * NPC:(core + 1) * NPC] = ob[:, :C]
        out[1, core * NPC:(core + 1) * NPC] = ob[:, C:]
    return out, res


def kernel(x, edge_index, f_disc_orig, fluxes, W_conc, W_disc, bias):
    out, _ = _run(dict(x=x, edge_index=edge_index, f_disc_orig=f_disc_orig,
                       fluxes=fluxes, W_conc=W_conc, W_disc=W_disc, bias=bias))
    return out


def profile_run(inputs):
    out, res = _run(inputs, trace=True)
    return res.exec_time_ns


# revision 20
# speedup vs baseline: 1.1483x; 1.0030x over previous
"""Trainium2 Bass kernel for nn_ReaReaConv (GCN-style message passing with
dynamic edge gating).

Math (per batch b):
    deg[n]   = in-degree(n) + 1 (self loop);  dis = rsqrt(deg)
    f_e      = keep*fdo + (1-keep)*(1-fdo), keep = sigmoid(2*flux[src]*flux[tgt])
    out[t]   = dis_t * ( (T-V)[t] @ Wc^T + V[t] @ Wd^T ) + bias
    T[t]     = sum_{e->t} dis_src * x[src_e]  + dis_t * x[t]   (self loop)
    V[t]     = sum_{e->t} dis_src * f_e * x[src_e]             (f_self = 0)

Sharding: each of the 8 cores owns N/8 target nodes (tiles of T=96). Host
sorts edges by (target tile, gather table, target) - indices/layout only;
all FP arithmetic runs on device. x is staged as a bf16 gather table (pure
byte truncation of fp32, a layout transform); edge-aggregation matmuls run
in bf16 with fp32 PSUM accumulation.

Key structure ("window one-hots"): within a tile the edges are target-sorted,
so each 128-edge chunk spans only a few consecutive targets. Every chunk gets
a uniform W=16 one-hot window (host picks the window base; splits a chunk in
the rare case its span exceeds W). Per super-tile, four merged DVE ops build
eq = is_equal(tl_rel, iota_W) and the three scaled copies eq*g, eq*(g*f0),
eq*(g*f1). Per chunk, three matmuls accumulate into PSUM column slices:
T += x_chunk @ (eq*g), V_b += x_chunk[b] @ (eq*g*f_b) (col-tiled M=64).
Self loops never enter the edge stream; a per-tile diagonal matmul adds
dis_t * x[t] to T (and zeroes the accumulator via start=True).

Gathers are batched per super-tile (3 tiles) into 4 dma_gather calls spread
over the 4 SWDGE queues. Per-tile/super-tile shapes are exact (maxed over the
8 cores) and baked into the program; builds are cached per shape signature.
"""

from dataclasses import dataclass

import ml_dtypes
import numpy as np

# -------------------- problem constants --------------------
N_NODES = 50000
N_EDGES = 1600000
BATCH = 2
C = 64
N_CORES = 8
TILE = 96            # target nodes per tile
W = 24               # one-hot window width per chunk
GSUP = 3             # tiles per super-tile (gather batching)
CHUNK = 128          # edges per matmul chunk (PE contraction)
SPLIT = 32768        # gather-table split (int16 signed index limit)

NPC = N_NODES // N_CORES          # nodes per core
NTC = -(-NPC // TILE)             # tiles per core (last one ragged)
NSUP = -(-NTC // GSUP)            # super-tiles per core


def bf16_trunc(a):
    """fp32 -> bf16 by dropping the low 16 bits (pure byte-layout transform)."""
    a = np.ascontiguousarray(np.asarray(a, np.float32))
    hi = (a.view(np.uint32) >> 16).astype(np.uint16)
    return hi.view(ml_dtypes.bfloat16)


@dataclass(frozen=True)
class Cfg:
    # per (super, tile-in-super) chunk counts, maxed over cores
    cta: tuple     # A-table chunks, flattened [NSUP*GSUP]
    ctb: tuple     # B-table chunks
    tn: tuple      # per-tile target count [NTC]
    base: tuple    # per chunk-slot window base, flattened in column order

    @property
    def sct(self):  # chunks per super-tile
        out = []
        for s in range(NSUP):
            sl = slice(s * GSUP, (s + 1) * GSUP)
            out.append(sum(self.cta[sl]) + sum(self.ctb[sl]))
        return tuple(out)

    @property
    def soff(self):  # super-tile column offsets
        o, acc = [], 0
        for c in self.sct:
            o.append(acc)
            acc += c
        return tuple(o)

    @property
    def ctn(self):
        return sum(self.sct)

    @property
    def na(self):
        return SPLIT

    @property
    def nb(self):
        return N_NODES - SPLIT


# -------------------- host prep (indices / layout only) --------------------

def _wrap16(idx_flat):
    """dma_gather index layout: [128, n/16] int16, idx[p, s] = flat[s*16+p],
    replicated across the 8 gpsimd cores (partition blocks of 16)."""
    n = len(idx_flat)
    assert n % 16 == 0
    w = np.asarray(idx_flat, np.int16).reshape(n // 16, 16).T
    return np.tile(w, (8, 1))


def _sorted_edges(edge_index):
    """Original edges only (no self loops), sorted by target."""
    src0 = np.asarray(edge_index[0]).astype(np.int64)
    tgt0 = np.asarray(edge_index[1]).astype(np.int64)
    perm = np.argsort(tgt0, kind="stable")
    return src0, tgt0, src0[perm], tgt0[perm], perm


def build_layout(edge_index):
    """Lockstep chunk schedule shared by all 8 cores.

    Per (tile, table-group), all cores pack their target-sorted edges into a
    common sequence of chunk slots.  Slot k's window base is the minimum
    "next target" over cores that still have edges (the laggard position);
    each core then takes up to CHUNK edges with target < base + W.  W absorbs
    both the per-chunk target span and the cross-core position spread, so
    slot counts stay within one of each core's own optimum.

    Returns (slots, assign):
      slots[tt] = (basesA, basesB) - window base per chunk slot
      assign[core][tt] = (selA, selB) - per slot, the selected positions into
          the core's tile-local edge order (indices into ga/gb subsets)
    """
    _, _, src_s, tgt_s, _ = _sorted_edges(edge_index)
    bounds = [tt * TILE for tt in range(NTC)] + [NPC]
    seg = []  # [core][tt] -> (ga, gb, tg) tile-local data
    for c in range(N_CORES):
        base = c * NPC
        starts = np.searchsorted(tgt_s, [base + b for b in bounds])
        tiles = []
        for tt in range(NTC):
            s, e = starts[tt], starts[tt + 1]
            seg_src = src_s[s:e]
            seg_tgt = tgt_s[s:e] - (base + tt * TILE)
            isa = seg_src < SPLIT
            ga = np.nonzero(isa)[0]
            gb = np.nonzero(~isa)[0]
            tiles.append((s, ga, seg_tgt[ga], gb, seg_tgt[gb]))
        seg.append(tiles)

    slots = []
    assign = [[None] * NTC for _ in range(N_CORES)]
    for tt in range(NTC):
        per_group = []
        sels = [[None, None] for _ in range(N_CORES)]
        for gi, (gsel_i, tg_i) in enumerate(((1, 2), (3, 4))):
            ptr = [0] * N_CORES
            tgs = [seg[c][tt][tg_i] for c in range(N_CORES)]
            bases = []
            core_sel = [[] for _ in range(N_CORES)]
            while True:
                nxt = [tgs[c][ptr[c]] for c in range(N_CORES)
                       if ptr[c] < len(tgs[c])]
                if not nxt:
                    break
                base = min(TILE - W, int(min(nxt)))
                bases.append(base)
                for c in range(N_CORES):
                    i0 = ptr[c]
                    j = min(i0 + CHUNK, len(tgs[c]))
                    hi = np.searchsorted(tgs[c][i0:j], base + W, side="left")
                    j = i0 + int(hi)
                    core_sel[c].append((i0, j))
                    ptr[c] = j
            per_group.append(tuple(bases))
            for c in range(N_CORES):
                sels[c][gi] = core_sel[c]
        slots.append((per_group[0], per_group[1]))
        for c in range(N_CORES):
            assign[c][tt] = (sels[c][0], sels[c][1])
    return seg, slots, assign


def prep(x, edge_index, f_disc_orig, fluxes):
    """Returns (cfg, shared dict, per-core dicts). Indices / layout /
    dtype-layout only - no floating-point arithmetic."""
    n = N_NODES
    src0, tgt0, src_s, tgt_s, perm = _sorted_edges(edge_index)
    x = np.asarray(x, np.float32)
    fdo_in = np.asarray(f_disc_orig, np.float32)
    fluxes = np.asarray(fluxes, np.float32)

    deg = (np.bincount(tgt0, minlength=n) + 1).astype(np.float32)

    per_edge_all = np.stack([
        fdo_in, fluxes[0][src0], fluxes[1][src0],
        fluxes[0][tgt0], fluxes[1][tgt0], deg[src0],
    ])  # [6, E]: fdo, fs0, fs1, ft0, ft1, degs
    per_edge = per_edge_all[:, perm]

    seg, slots, assign = build_layout(edge_index)

    # column layout: per super-tile: [A(t0) A(t1) A(t2) B(t0) B(t1) B(t2)]
    cta = [len(slots[tt][0]) for tt in range(NTC)]
    ctb = [len(slots[tt][1]) for tt in range(NTC)]
    tn = tuple(min(TILE, NPC - tt * TILE) for tt in range(NTC))
    col_of = {}
    bases = []
    acc = 0
    for s in range(NSUP):
        tset = range(s * GSUP, min((s + 1) * GSUP, NTC))
        for grp, gi in (("a", 0), ("b", 1)):
            for tt in tset:
                col_of[(tt, grp)] = acc
                bases.extend(slots[tt][gi])
                acc += len(slots[tt][gi])
    ctn = acc
    cfg = Cfg(cta=tuple(cta), ctb=tuple(ctb), tn=tn, base=tuple(bases))

    xcat = np.concatenate([x[0], x[1]], axis=1)  # [n, 128]
    iota_w = np.tile(np.arange(W, dtype=np.float32), (128, 1))
    id96 = np.zeros((128, TILE), np.float32)
    id96[np.arange(TILE), np.arange(TILE)] = 1.0
    shared = {
        "xpa": bf16_trunc(xcat[:SPLIT]),
        "xpb": bf16_trunc(xcat[SPLIT:]),
        "iotaw": bf16_trunc(iota_w),
        "id96": bf16_trunc(id96),
        "zeros96": np.zeros((128, TILE), ml_dtypes.bfloat16),
        "zeros128": np.zeros((128, 128), ml_dtypes.bfloat16),
    }

    names = ["fdo", "fs0", "fs1", "ft0", "ft1"]
    cores = []
    for core in range(N_CORES):
        base_node = core * NPC
        tl_all = np.full((128, ctn), -1.0, np.float32)
        pe_all = np.zeros((6, 128, ctn), np.float32)
        pe_all[5] = 1.0
        idx16 = np.zeros((128, ctn * 8), np.int16)
        degown = np.ones((128, NTC), np.float32)
        degrow = np.ones((1, NTC * TILE), np.float32)
        xq = np.zeros((128, NTC * 128), np.float32)
        for tt in range(NTC):
            s, ga, tga, gb, tgb = seg[core][tt]
            selA, selB = assign[core][tt]
            for grp, gsel, tg, chunks, idoff in (
                    ("a", ga, tga, selA, 0), ("b", gb, tgb, selB, SPLIT)):
                c0 = col_of[(tt, grp)]
                sbases = slots[tt][0 if grp == "a" else 1]
                for k, (i0, i1) in enumerate(chunks):
                    nn = i1 - i0
                    if nn == 0:
                        continue
                    col = c0 + k
                    b = sbases[k]
                    sel = gsel[i0:i1]
                    tl_all[:nn, col] = tg[i0:i1] - b
                    for j in range(6):
                        pe_all[j][:nn, col] = per_edge[j][s + sel]
                    pe_all[5][nn:, col] = 1.0
                    ids = np.zeros(CHUNK, np.int64)
                    ids[:nn] = src_s[s + sel] - idoff
                    idx16[:, col * 8:(col + 1) * 8] = _wrap16(ids)

            tnn = tn[tt]
            t0g = base_node + tt * TILE
            degown[:tnn, tt] = deg[t0g:t0g + tnn]
            degrow[0, tt * TILE:tt * TILE + tnn] = deg[t0g:t0g + tnn]
            xq[:tnn, tt * 128:tt * 128 + 128] = xcat[t0g:t0g + tnn]
        d = {"tl": bf16_trunc(tl_all), "idx16": idx16,
             "degown": degown, "degrow": degrow,
             "degs": np.ascontiguousarray(pe_all[5]),
             "xq": bf16_trunc(xq)}
        for j, nm in enumerate(names):
            d[nm] = bf16_trunc(pe_all[j])
        cores.append(d)
    return cfg, shared, cores


# -------------------- device program --------------------

def build_nc(cfg: Cfg):
    import concourse.tile as tile
    from concourse import bacc, mybir

    dt = mybir.dt
    act = mybir.ActivationFunctionType
    alu = mybir.AluOpType

    ctn = cfg.ctn
    T = TILE

    nc = bacc.Bacc("TRN2", target_bir_lowering=False, debug=False,
                   num_swdge_queues=4)

    xpa = nc.dram_tensor("xpa", [cfg.na, 2 * C], dt.bfloat16, kind="ExternalInput")
    xpb = nc.dram_tensor("xpb", [cfg.nb, 2 * C], dt.bfloat16, kind="ExternalInput")
    tl_d = nc.dram_tensor("tl", [128, ctn], dt.bfloat16, kind="ExternalInput")
    fdo_d = nc.dram_tensor("fdo", [128, ctn], dt.bfloat16, kind="ExternalInput")
    fs0_d = nc.dram_tensor("fs0", [128, ctn], dt.bfloat16, kind="ExternalInput")
    fs1_d = nc.dram_tensor("fs1", [128, ctn], dt.bfloat16, kind="ExternalInput")
    ft0_d = nc.dram_tensor("ft0", [128, ctn], dt.bfloat16, kind="ExternalInput")
    ft1_d = nc.dram_tensor("ft1", [128, ctn], dt.bfloat16, kind="ExternalInput")
    degs_d = nc.dram_tensor("degs", [128, ctn], dt.float32, kind="ExternalInput")
    idx16_d = nc.dram_tensor("idx16", [128, ctn * 8], dt.int16, kind="ExternalInput")
    degown_d = nc.dram_tensor("degown", [128, NTC], dt.float32, kind="ExternalInput")
    degrow_d = nc.dram_tensor("degrow", [1, NTC * T], dt.float32, kind="ExternalInput")
    iotaw_d = nc.dram_tensor("iotaw", [128, W], dt.bfloat16, kind="ExternalInput")
    id96_d = nc.dram_tensor("id96", [128, T], dt.bfloat16, kind="ExternalInput")
    z96_d = nc.dram_tensor("zeros96", [128, T], dt.bfloat16, kind="ExternalInput")
    z128_d = nc.dram_tensor("zeros128", [128, 128], dt.bfloat16, kind="ExternalInput")
    xq_d = nc.dram_tensor("xq", [128, NTC * 128], dt.bfloat16, kind="ExternalInput")
    wct_d = nc.dram_tensor("wct2", [128, C], dt.bfloat16, kind="ExternalInput")
    wdt_d = nc.dram_tensor("wdt2", [128, C], dt.bfloat16, kind="ExternalInput")
    bias_d = nc.dram_tensor("bias2", [1, 2 * C], dt.bfloat16, kind="ExternalInput")
    outB = nc.dram_tensor("outB", [NPC, 2 * C], dt.float32, kind="ExternalOutput")

    with tile.TileContext(nc) as tc:
        with (
            tc.tile_pool(name="const", bufs=1) as constp,
            tc.tile_pool(name="res", bufs=1) as resp,
        ):
            iotaw_sb = constp.tile([128, W], dt.bfloat16)
            nc.sync.dma_start(iotaw_sb[:], iotaw_d[:, :])
            id96_sb = constp.tile([128, T], dt.bfloat16)
            nc.sync.dma_start(id96_sb[:], id96_d[:, :])
            z96_sb = constp.tile([128, T], dt.bfloat16)
            nc.sync.dma_start(z96_sb[:], z96_d[:, :])
            z128_sb = constp.tile([128, 128], dt.bfloat16)
            nc.sync.dma_start(z128_sb[:], z128_d[:, :])
            wct_sb = constp.tile([128, C], dt.bfloat16)
            nc.sync.dma_start(wct_sb[:], wct_d[:, :])
            wdt_sb = constp.tile([128, C], dt.bfloat16)
            nc.sync.dma_start(wdt_sb[:], wdt_d[:, :])
            bias_sb = constp.tile([1, 2 * C], dt.bfloat16)
            nc.sync.dma_start(bias_sb[:], bias_d[:, :])

            disown_sb = constp.tile([128, NTC], dt.float32)
            nc.sync.dma_start(disown_sb[:], degown_d[:, :])
            nc.vector.reciprocal(disown_sb[:], disown_sb[:])
            nc.scalar.activation(disown_sb[:], disown_sb[:], act.Sqrt)
            sqdeg_sb = constp.tile([1, NTC * T], dt.bfloat16)
            sqf_sb = constp.tile([1, NTC * T], dt.float32)
            nc.sync.dma_start(sqf_sb[:], degrow_d[:, :])
            nc.scalar.activation(sqdeg_sb[:], sqf_sb[:], act.Sqrt)

            dxall = resp.tile([128, NTC * T], dt.bfloat16)
            # warm-up gather: loads the Q7 gather ucode + inits rings while
            # the input DMAs stream, off the critical path
            with tc.high_priority():
                wuidx = resp.tile([128, 8], dt.int16)
                nc.gpsimd.memset(wuidx[:], 0)
                for q in range(4):
                    wug = resp.tile([128, 2 * C], dt.bfloat16, tag=f"wug{q}",
                                    name=f"wug{q}")
                    nc.gpsimd.dma_gather(
                        wug[:].rearrange("p (c r) -> p c r", r=2 * C),
                        xpa[:, :], wuidx[:], CHUNK, CHUNK, 2 * C,
                        single_packet=False, queue_num=q)
                # idx per super-tile as separate tiles, so each super's
                # gathers depend only on their own slice's load; priority 0
                # puts these DMAs ahead of the bulk metadata loads
                _soff = cfg.soff
                _sct = cfg.sct
                idx_tiles = []
                for _s in range(NSUP):
                    it = resp.tile([128, _sct[_s] * 8], dt.int16,
                                   tag=f"idx{_s}", name=f"idx{_s}")
                    sl = idx16_d[:, _soff[_s] * 8:(_soff[_s] + _sct[_s]) * 8]
                    if _s < 2:
                        # SWDGE path: lands immediately, independent of the
                        # metadata flood on the sync DMA queue
                        nc.gpsimd.dma_start(it[:], sl)
                    else:
                        nc.sync.dma_start(it[:], sl)
                    idx_tiles.append(it)
            tl_sb = resp.tile([128, ctn], dt.bfloat16)
            nc.sync.dma_start(tl_sb[:], tl_d[:, :])
            xq_sb = resp.tile([128, NTC * 128], dt.bfloat16)
            nc.sync.dma_start(xq_sb[:], xq_d[:, :])
            g_sb = resp.tile([128, ctn], dt.bfloat16)
            gf0_sb = resp.tile([128, ctn], dt.bfloat16)
            gf1_sb = resp.tile([128, ctn], dt.bfloat16)

            # ---- prepass: g = rsqrt(deg_src), gf_b = g * f_b ----
            with tc.tile_pool(name="pp", bufs=1) as ppp:
                grec = ppp.tile([128, ctn], dt.float32)
                nc.sync.dma_start(grec[:], degs_d[:, :])
                nc.vector.reciprocal(grec[:], grec[:])
                nc.scalar.activation(g_sb[:], grec[:], act.Sqrt)

                fdo_sb = ppp.tile([128, ctn], dt.bfloat16)
                nc.sync.dma_start(fdo_sb[:], fdo_d[:, :])
                c1 = ppp.tile([128, ctn], dt.bfloat16)
                nc.vector.tensor_scalar(
                    c1[:], fdo_sb[:], 2.0, -1.0, alu.mult, alu.add)
                c0 = ppp.tile([128, ctn], dt.bfloat16)
                nc.vector.tensor_scalar(
                    c0[:], fdo_sb[:], -1.0, 1.0, alu.mult, alu.add)
                for b, (fsd, ftd, gfd) in enumerate(
                        ((fs0_d, ft0_d, gf0_sb), (fs1_d, ft1_d, gf1_sb))):
                    fs_sb = ppp.tile([128, ctn], dt.bfloat16, tag="fs")
                    nc.sync.dma_start(fs_sb[:], fsd[:, :])
                    ft_sb = ppp.tile([128, ctn], dt.bfloat16, tag="ft")
                    nc.sync.dma_start(ft_sb[:], ftd[:, :])
                    nc.vector.tensor_mul(fs_sb[:], fs_sb[:], ft_sb[:])
                    nc.scalar.activation(
                        ft_sb[:], fs_sb[:], act.Sigmoid, scale=2.0)
                    nc.vector.tensor_mul(ft_sb[:], ft_sb[:], c1[:])
                    nc.vector.tensor_add(ft_sb[:], ft_sb[:], c0[:])
                    nc.vector.tensor_mul(gfd[:], ft_sb[:], g_sb[:])
                for tt in range(NTC):
                    nc.vector.tensor_scalar(
                        dxall[:T, tt * T:(tt + 1) * T], id96_sb[:T, :],
                        disown_sb[:T, tt:tt + 1], None, alu.mult)

            # ---- main loop over super-tiles / tiles ----
            with (
                tc.tile_pool(name="xg", bufs=3) as xgp,
                tc.tile_pool(name="oe", bufs=2) as oep,
                tc.tile_pool(name="uv", bufs=4) as uvp,
                tc.tile_pool(name="outp", bufs=4) as outsp,
                tc.tile_pool(name="ps_tv", bufs=3, space="PSUM") as pstv,
                tc.tile_pool(name="ps_o", bufs=2, space="PSUM") as pso,
            ):
                qrr = 0
                soff = cfg.soff
                for s in range(NSUP):
                    tset = list(range(s * GSUP, min((s + 1) * GSUP, NTC)))
                    sct = cfg.sct[s]
                    scta = sum(cfg.cta[tt] for tt in tset)
                    sc0 = soff[s]
                    ib = sc0 * 8

                    xg = xgp.tile([128, sct * 2 * C], dt.bfloat16, tag="xg")
                    xg3 = xg[:].rearrange("p (c r) -> p c r", r=2 * C)
                    calls = [(0, scta // 2, xpa), (scta // 2, scta, xpa),
                             (scta, scta + (sct - scta) // 2, xpb),
                             (scta + (sct - scta) // 2, sct, xpb)]
                    for (c0_, c1_, tbl) in calls:
                        nch = c1_ - c0_
                        if nch == 0:
                            continue
                        nc.gpsimd.dma_gather(
                            xg3[:, c0_:c1_],
                            tbl[:, :],
                            idx_tiles[s][:, c0_ * 8: c1_ * 8],
                            nch * CHUNK, nch * CHUNK, 2 * C,
                            single_packet=False, queue_num=qrr % 4,
                        )
                        qrr += 1

                    # merged window one-hots for the whole super-tile
                    eqt = oep.tile([128, sct * W], dt.bfloat16, tag="eq")
                    e3 = eqt[:].rearrange("p (c w) -> p c w", w=W)
                    tl_cols = tl_sb[:, sc0:sc0 + sct].unsqueeze(2)
                    nc.vector.tensor_tensor(
                        e3,
                        tl_cols.to_broadcast([128, sct, W]),
                        iotaw_sb[:].unsqueeze(1).to_broadcast([128, sct, W]),
                        alu.is_equal,
                    )
                    ohg = oep.tile([128, sct * W], dt.bfloat16, tag="ohg")
                    ohf0 = oep.tile([128, sct * W], dt.bfloat16, tag="ohf0")
                    ohf1 = oep.tile([128, sct * W], dt.bfloat16, tag="ohf1")
                    for dst, src_cols in ((ohg, g_sb), (ohf0, gf0_sb),
                                          (ohf1, gf1_sb)):
                        d3 = dst[:].rearrange("p (c w) -> p c w", w=W)
                        sc = src_cols[:, sc0:sc0 + sct].unsqueeze(2)
                        nc.vector.tensor_tensor(
                            d3, e3, sc.to_broadcast([128, sct, W]), alu.mult)

                    for tt in tset:
                        tnn = cfg.tn[tt]
                        # chunk columns of this tile within the super layout
                        ca, cb = cfg.cta[tt], cfg.ctb[tt]
                        a0 = sum(cfg.cta[t2] for t2 in tset if t2 < tt)
                        b0 = scta + sum(cfg.ctb[t2] for t2 in tset if t2 < tt)
                        cols = ([a0 + k for k in range(ca)] +
                                [b0 + k for k in range(cb)])

                        t_ps = pstv.tile([128, T], dt.float32, tag="t_ps")
                        v_ps = pstv.tile([128, T], dt.float32, tag="v_ps")

                        # diagonal self-loop matmul zeroes + seeds T:
                        # dx[p, t] = delta(p==t) * dis_t (prebuilt)
                        nc.tensor.matmul(
                            out=t_ps[:],
                            lhsT=xq_sb[:T, tt * 128:(tt + 1) * 128],
                            rhs=dxall[:T, tt * T:(tt + 1) * T],
                            start=True, stop=False)
                        # V accumulator: zero it with a zeros matmul
                        nc.tensor.matmul(
                            out=v_ps[:], lhsT=z128_sb[:], rhs=z96_sb[:],
                            start=True, stop=False)

                        for ci, col in enumerate(cols):
                            base = cfg.base[sc0 + col]
                            lsl = xg[:, col * 2 * C:(col + 1) * 2 * C]
                            wsl = slice(col * W, (col + 1) * W)
                            psl = slice(base, base + W)
                            nc.tensor.matmul(
                                out=t_ps[:, psl], lhsT=lsl, rhs=ohg[:, wsl],
                                start=False, stop=(ci == len(cols) - 1))
                            nc.tensor.matmul(
                                out=v_ps[0:64, psl], lhsT=lsl[:, 0:C],
                                rhs=ohf0[:, wsl], start=False, stop=False,
                                tile_position=(0, 0))
                            nc.tensor.matmul(
                                out=v_ps[64:128, psl], lhsT=lsl[:, C:2 * C],
                                rhs=ohf1[:, wsl], start=False,
                                stop=(ci == len(cols) - 1),
                                tile_position=(0, 64))

                        # epilogue
                        vm = uvp.tile([128, T], dt.bfloat16, tag="vm")
                        nc.scalar.activation(vm[:], v_ps[:], act.Copy)
                        um = uvp.tile([128, T], dt.bfloat16, tag="um")
                        nc.vector.tensor_tensor(um[:], t_ps[:], vm[:],
                                                alu.subtract)

                        op_ps = pso.tile([T, 2 * C], dt.float32, tag="op")
                        sq_row = sqdeg_sb[0:1, tt * T: tt * T + T]
                        for bi in range(2):
                            rows = slice(C * bi, C * bi + C)
                            csl = slice(C * bi, C * bi + C)
                            nc.tensor.matmul(
                                out=op_ps[:, csl], lhsT=um[rows, :],
                                rhs=wct_sb[rows, :], start=True, stop=False)
                            nc.tensor.matmul(
                                out=op_ps[:, csl], lhsT=vm[rows, :],
                                rhs=wdt_sb[rows, :], start=False, stop=False)
                            nc.tensor.matmul(
                                out=op_ps[:, csl], lhsT=sq_row,
                                rhs=bias_sb[0:1, csl], start=False, stop=True)

                        o_sb = outsp.tile([128, 2 * C], dt.float32, tag="os")
                        nc.vector.tensor_scalar(
                            o_sb[:T, :], op_ps[:, :], disown_sb[:T, tt:tt + 1],
                            None, alu.mult)
                        nc.sync.dma_start(
                            outB[tt * T:tt * T + tnn, :], o_sb[:tnn, :])

    nc.compile()
    return nc


def _shared_weights(W_conc, W_disc, bias):
    wct2 = np.zeros((128, C), np.float32)
    wdt2 = np.zeros((128, C), np.float32)
    wct2[:64] = np.asarray(W_conc, np.float32).T
    wct2[64:] = wct2[:64]
    wdt2[:64] = np.asarray(W_disc, np.float32).T
    wdt2[64:] = wdt2[:64]
    bias2 = np.tile(np.asarray(bias, np.float32)[None, :], (1, 2))
    return bf16_trunc(wct2), bf16_trunc(wdt2), bf16_trunc(bias2)


_NC_CACHE = {}


def _run(inputs, trace=False):
    from concourse.bass_utils import run_bass_kernel_spmd

    x = np.asarray(inputs["x"], np.float32)
    cfg, shared, cores = prep(x, inputs["edge_index"],
                              inputs["f_disc_orig"], inputs["fluxes"])
    wct2, wdt2, bias2 = _shared_weights(
        inputs["W_conc"], inputs["W_disc"], inputs["bias"])
    in_maps = []
    for core in range(N_CORES):
        m = dict(shared)
        m.update(cores[core])
        m["wct2"] = wct2
        m["wdt2"] = wdt2
        m["bias2"] = bias2
        in_maps.append(m)

    if cfg not in _NC_CACHE:
        _NC_CACHE[cfg] = build_nc(cfg)
    nc = _NC_CACHE[cfg]

    res = run_bass_kernel_spmd(nc, in_maps, list(range(N_CORES)),
                               trace=trace)
    out = np.zeros((BATCH, N_NODES, C), np.float32)
    for core in range(N_CORES):
        ob = res.results[core]["outB"]
        out[0, core * NPC:(core + 1) * NPC] = ob[:, :C]
        out[1, core * NPC:(core + 1) * NPC] = ob[:, C:]
    return out, res


def kernel(x, edge_index, f_disc_orig, fluxes, W_conc, W_disc, bias):
    out, _ = _run(dict(x=x, edge_index=edge_index, f_disc_orig=f_disc_orig,
                       fluxes=fluxes, W_conc=W_conc, W_disc=W_disc, bias=bias))
    return out


def profile_run(inputs):
    out, res = _run(inputs, trace=True)
    return res.exec_time_ns
